# revision 34
# baseline (speedup 1.0000x reference)
"""Trainium2 Bass kernel for nn_Conv_39273180955618.

The reference op reduces to a depthwise correlation: every image (batch x
channel plane) of X is correlated with the same 3x3 kernel
Keff = K.sum((0,1)), plus a scalar bias b * prod(K.shape).

Strategy (8 NeuronCores, data-parallel over batch):
  - core k gets batches [2k, 2k+2) = 128 images of 224x224.
  - All device I/O is fp16 (the 2e-2 rel-err budget dwarfs fp16 rounding),
    halving HBM traffic vs fp32. DRAM tensors are laid out [row, img, col]
    so each DMA descriptor is a 7-14 KB contiguous run (>=512 B keeps the
    DMA bus at full rate; smaller runs are charged double).
  - Per core, images are processed in blocks of IB images x 112-row chunks.
    Rows live on SBUF partitions, W stays contiguous on the free axis.
  - The H-convolution is a TensorE matmul contraction over rows with small
    banded matrices B[chunk, dw] (shape [113, 112]): for each of the 3 W
    shifts dw, Z[:, wout] += B^T @ X[rows, win], accumulated in PSUM.
    H zero-padding is folded into the band matrices, W zero-padding into
    the matmul column ranges. fp16 matmuls run 1 row/cycle at 2.4 GHz.
  - All X tiles are SBUF-resident, so every load issues up front and the
    DMA engines stream input at full rate decoupled from PE pace; stores
    ride the otherwise-idle GPSIMD SWDGE ring so their waits never block
    the eviction engines' sequencers.
  - PSUM -> SBUF eviction (+ bias, fp32->fp16) alternates ScalarE/VectorE.
"""

import numpy as np

import bass_rust
import concourse.bass as bass
import concourse.mybir as mybir
import concourse.tile as tile
from concourse.bass_utils import run_bass_kernel_spmd

F16 = mybir.dt.float16
F32 = mybir.dt.float32

N_CORES = 8
H = W = 224
M = 112        # output rows per chunk
KR = 113       # input rows per chunk (M + 1 halo row at the image edge)
IMGS = 128     # images per core (2 batches x 64 channels)
IB = 32        # images per block (DMA granularity)
NBLK = IMGS // IB
WP = W + 2     # padded image-row width in SBUF (zero column at each edge)
# (r0, i0) per chunk: output-row base and input-row base.
CHUNKS = ((0, 0), (112, 111))

_MAX_WAITS = 1


def _split_multi_waits(nc):
    """Split instructions carrying >1 sync-wait into single-wait NOP
    preludes (the walrus build here rejects multi-wait instructions)."""
    counter = 0
    for fn in nc.m.functions:
        for bb in fn.blocks:
            insts = bb.instructions
            i = 0
            while i < len(insts):
                inst = insts[i]
                si = inst.sync_info
                if si is not None and si.on_wait and len(si.on_wait) > _MAX_WAITS:
                    waits = list(si.on_wait)
                    keep = waits[-_MAX_WAITS:]
                    spill = waits[:-_MAX_WAITS]
                    nops = []
                    for w in spill:
                        nop = mybir.InstNoOp(
                            name=f"waitsplit_{counter}", ins=[], outs=[]
                        )
                        counter += 1
                        nop.engine = inst.engine
                        nop.sync_info = bass_rust.SyncInfo(on_wait=[w], on_update=[])
                        nops.append(nop)
                    inst.sync_info = bass_rust.SyncInfo(
                        on_wait=keep,
                        on_update=list(si.on_update) if si.on_update else [],
                    )
                    insts[i:i] = nops
                    i += len(nops)
                i += 1
    return counter


def build_nc(bias_total: float):
    nc = bass.Bass("TRN2", target_bir_lowering=False, debug=False)
    x_d = nc.dram_tensor("X", [H, IMGS, WP], F16, kind="ExternalInput").ap()
    bands_d = nc.dram_tensor("BANDS", [KR, 2, 3, M], F16, kind="ExternalInput").ap()
    y_d = nc.dram_tensor("Y", [H, IMGS, W], F16, kind="ExternalOutput").ap()

    with tile.TileContext(nc) as tc:
        with (
            tc.tile_pool(name="const", bufs=1) as cpool,
            tc.tile_pool(name="io", bufs=5) as io_pool,
            tc.tile_pool(name="acc", bufs=8, space="PSUM") as psum_pool,
        ):
            # Chunk 0's bands load first (211 ns); chunk 1's bands are
            # deferred below the first image loads (not needed until ~14 us
            # in), keeping the head's serial HWDGE chain short.
            bands = cpool.tile([KR, 2, 3, M], F16)
            nc.sync.dma_start(bands[:, 0:1, :, :], bands_d[:, 0:1, :, :])
            ev = 0
            for blk in range(NBLK):
                for c, (r0, i0) in enumerate(CHUNKS):
                    # Enough xt bufs that loads never wait on recycling:
                    # all loads issue immediately and the DMA engines
                    # stream the whole input up front, decoupled from PE
                    # pace. X arrives host-padded to 226 columns (zero at
                    # each edge) and host-transposed to [row, img, col], so
                    # each partition's 16-image row block is one contiguous
                    # 7.2 KB descriptor. The very first load lands in small
                    # pieces so the first matmuls start as early as possible
                    # (the PE p-state ramps on real work while the rest of
                    # the input streams in).
                    xt = io_pool.tile([KR, IB, WP], F16, tag="xt", bufs=6)
                    splits = (2, 4, 4, 6, 8, 8) if (blk, c) == (0, 0) else (16, 16)
                    h0i = 0
                    for sz in splits:
                        nc.sync.dma_start(
                            xt[:, h0i:h0i + sz, :],
                            x_d[
                                i0:i0 + KR,
                                blk * IB + h0i:blk * IB + h0i + sz,
                                :,
                            ],
                        )
                        h0i += sz
                    if (blk, c) == (0, 0):
                        nc.sync.dma_start(bands[:, 1:2, :, :], bands_d[:, 1:2, :, :])
                    ot = io_pool.tile([M, IB, W], F16, tag="ot", bufs=6)
                    for p in range(IB // 2):
                        # One 2D-windowed matmul per W-shift: the rhs free AP
                        # is [2 images, 224 cols] shifted by dw within each
                        # padded 226-col row, and the PSUM dst is a dense
                        # [2, 224] pair block -- no junk columns streamed.
                        ps = psum_pool.tile([M, 2 * W], F32)
                        for k, dw in enumerate((0, 1, 2)):
                            nc.tensor.matmul(
                                ps.rearrange("m (i w) -> m i w", w=W),
                                bands[:, c, dw, :],
                                xt[:, 2 * p:2 * p + 2, dw:dw + W],
                                start=(k == 0),
                                stop=(k == 2),
                            )
                        psv = ps.rearrange("m (i w) -> m i w", w=W)
                        dst = ot[:, 2 * p:2 * p + 2, :]
                        if blk == NBLK - 1 and c == 1 and p == IB // 2 - 1:
                            # Split the very last eviction across both
                            # engines so the final store fires sooner.
                            if bias_total != 0.0:
                                nc.scalar.activation(
                                    dst[:, 0:1, :], psv[:, 0:1, :],
                                    mybir.ActivationFunctionType.Copy,
                                    bias=float(bias_total),
                                )
                                nc.vector.tensor_scalar_add(
                                    dst[:, 1:2, :], psv[:, 1:2, :],
                                    float(bias_total),
                                )
                            else:
                                nc.scalar.copy(dst[:, 0:1, :], psv[:, 0:1, :])
                                nc.vector.tensor_copy(dst[:, 1:2, :], psv[:, 1:2, :])
                            ev += 1
                        elif ev % 2 == 0:
                            if bias_total != 0.0:
                                nc.scalar.activation(
                                    dst,
                                    psv,
                                    mybir.ActivationFunctionType.Copy,
                                    bias=float(bias_total),
                                )
                            else:
                                nc.scalar.copy(dst, psv)
                        else:
                            if bias_total != 0.0:
                                nc.vector.tensor_scalar_add(
                                    dst, psv, float(bias_total)
                                )
                            else:
                                nc.vector.tensor_copy(dst, psv)
                        ev += 1
                        # Stores ride the GPSIMD SWDGE ring: it is
                        # otherwise idle, so store waits never block the
                        # eviction engines' sequencers, and SWDGE bypasses
                        # the shared HWDGE descriptor generator. 8-image
                        # stores keep each descriptor at 3.5 KB contiguous.
                        # The last half-block instead drains through small
                        # stores: 4 images via SWDGE after pair 13, then two
                        # 2-image stores on the (by then empty) SP/HWDGE
                        # ring, which restarts ~1.7 us faster per op than
                        # SWDGE, so the post-compute tail is as short as the
                        # store pipeline allows.
                        last8 = blk == NBLK - 1 and c == 1 and p >= IB // 2 - 4
                        if last8:
                            if p == 13:
                                nc.gpsimd.dma_start(
                                    y_d[r0:r0 + M, blk * IB + 24:blk * IB + 28, :],
                                    ot[:, 24:28, :],
                                )
                            elif p >= 14:
                                h0 = 2 * p
                                nc.sync.dma_start(
                                    y_d[r0:r0 + M, blk * IB + h0:blk * IB + h0 + 2, :],
                                    ot[:, h0:h0 + 2, :],
                                )
                        elif p % 4 == 3:
                            h0 = (p - 3) * 2
                            nc.gpsimd.dma_start(
                                y_d[
                                    r0:r0 + M,
                                    blk * IB + h0:blk * IB + h0 + 8,
                                    :,
                                ],
                                ot[:, h0:h0 + 8, :],
                            )
    _split_multi_waits(nc)
    return nc


def build_bands(Keff: np.ndarray) -> np.ndarray:
    """Banded H-contraction matrices, [KR, chunk, dw, M] fp16.

    B[i, c, dw, m] = Keff[dh, dw] where input-row index i corresponds to
    absolute row i0 + i and output row r0 + m needs absolute row
    r0 + m + dh - 1; rows outside [0, H) are dropped (zero padding).
    """
    bands = np.zeros((KR, 2, 3, M), dtype=np.float32)
    for c, (r0, i0) in enumerate(CHUNKS):
        for dw in range(3):
            for m in range(M):
                for dh in range(3):
                    arow = r0 + m + dh - 1
                    if 0 <= arow < H:
                        bands[arow - i0, c, dw, m] = Keff[dh, dw]
    return bands.astype(np.float16)


_cache = {}


def kernel(X, K, b, padding, stride) -> np.ndarray:
    X = np.ascontiguousarray(np.asarray(X, dtype=np.float32))
    K = np.asarray(K, dtype=np.float32)
    b = np.asarray(b, dtype=np.float32)
    assert int(padding) == 1 and int(stride) == 1, (padding, stride)
    bx, cx, hx, wx = X.shape
    assert (bx, cx, hx, wx) == (16, 64, H, W), X.shape

    bk, ck, hk, wk = K.shape
    Keff = K.sum(axis=(0, 1), dtype=np.float32)
    bias_total = float(b.reshape(())) * (bk * ck * hk * wk)

    key = round(bias_total, 12)
    if key not in _cache:
        _cache[key] = build_nc(bias_total)
    nc = _cache[key]

    bands = build_bands(Keff)
    Xh = X.reshape(bx * cx, hx, wx).astype(np.float16)
    in_maps = []
    for k in range(N_CORES):
        Xp = np.zeros((H, IMGS, WP), dtype=np.float16)
        Xp[:, :, 1:1 + W] = Xh[k * IMGS:(k + 1) * IMGS].transpose(1, 0, 2)
        in_maps.append({"X": Xp, "BANDS": bands})
    res = run_bass_kernel_spmd(nc, in_maps, core_ids=list(range(N_CORES)))
    out = np.empty((bx * cx, H, W), dtype=np.float32)
    for k, r in enumerate(res.results):
        out[k * IMGS:(k + 1) * IMGS] = r["Y"].transpose(1, 0, 2)
    return out.reshape(bx, cx, hx, wx)

